# revision 4
# baseline (speedup 1.0000x reference)
"""BP-MLL loss kernel for Trainium2 (Bass/Tile), data-parallel over 8 NeuronCores.

Reference computation (per row r of [B, L] inputs):
    s_pos[r] = sum_{j: t=1} exp(-x[r,j])
    s_neg[r] = sum_{j: t=0} exp( x[r,j])
    n_pos[r] = #{j: t=1},  n_neg[r] = L - n_pos[r]
    loss     = sum_r s_pos[r]*s_neg[r] / (n_pos[r]*n_neg[r])

Sharding: batch dim B=8192 split 8 ways (1024 rows/core); each core computes a
scalar partial loss on-device; host sums the 8 partials.

Per-core device plan, for each [128 rows, F cols] tile (rows on partitions):
  ACT:    e_neg = exp(-x)
  ACT:    e_pos = exp(x), fused accum -> sum_all = sum_j exp(x)
  DVE:    scalar_tensor_tensor (t*1)*e_neg, fused accum -> s_pos
  DVE:    scalar_tensor_tensor (t*1)*e_pos, fused accum -> se_pos
          (s_neg = sum_all - se_pos)
  GPSIMD: tensor_scalar t*1, fused accum -> n_pos
Epilogue: per-row arithmetic on [128, 8] partials, then a (-1)-weighted
matmul reduces 128 partitions to the scalar partial loss.
"""

import numpy as np

import concourse.bacc as bacc
import concourse.bass as bass
import concourse.tile as tile
from concourse import mybir
from concourse.bass_utils import run_bass_kernel_spmd

F32 = mybir.dt.float32
I32 = mybir.dt.int32
AF = mybir.ActivationFunctionType
ALU = mybir.AluOpType

B, L = 8192, 10000
N_CORES = 8
ROWS = B // N_CORES  # rows per core
P = 128


def build_bass(rows=ROWS, cols=L, f_c=2500, npos_engine="act"):
    """Build the per-core Bass program. Same program runs SPMD on all cores."""
    assert rows % P == 0 and cols % f_c == 0
    n_rg = rows // P
    n_ch = cols // f_c

    nc = bacc.Bacc("TRN2", target_bir_lowering=False, debug=False)
    x = nc.dram_tensor("x", [rows, cols], F32, kind="ExternalInput").ap()
    t = nc.dram_tensor("t", [rows, cols], I32, kind="ExternalInput").ap()
    out = nc.dram_tensor("out", [1, 1], F32, kind="ExternalOutput").ap()

    with tile.TileContext(nc) as tc:
        with (
            tc.tile_pool(name="io", bufs=3) as io_pool,
            tc.tile_pool(name="work", bufs=2) as work_pool,
            tc.tile_pool(name="acc", bufs=1) as acc_pool,
            tc.tile_pool(name="small", bufs=1) as small_pool,
            tc.tile_pool(name="psum", bufs=1, space="PSUM") as psum_pool,
        ):
            n_slots = n_rg * n_ch
            acc_spos = acc_pool.tile([P, n_slots], F32, tag="acc_spos")
            acc_sepos = acc_pool.tile([P, n_slots], F32, tag="acc_sepos")
            acc_sall = acc_pool.tile([P, n_slots], F32, tag="acc_sall")
            acc_npos = acc_pool.tile([P, n_slots], F32, tag="acc_npos")

            for rg in range(n_rg):
                for c in range(n_ch):
                    sl = rg * n_ch + c
                    r0 = rg * P
                    c0 = c * f_c
                    xt = io_pool.tile([P, f_c], F32, tag="x")
                    tt = io_pool.tile([P, f_c], I32, tag="t")
                    nc.sync.dma_start(xt[:], x[r0 : r0 + P, c0 : c0 + f_c])
                    nc.sync.dma_start(tt[:], t[r0 : r0 + P, c0 : c0 + f_c])

                    e_neg = work_pool.tile([P, f_c], F32, tag="eneg")
                    e_pos = work_pool.tile([P, f_c], F32, tag="epos")
                    prod = work_pool.tile([P, f_c], F32, tag="prod")

                    nc.scalar.activation(e_neg[:], xt[:], AF.Exp, scale=-1.0)
                    nc.scalar.activation(
                        e_pos[:],
                        xt[:],
                        AF.Exp,
                        scale=1.0,
                        accum_out=acc_sall[:, sl : sl + 1],
                    )
                    nc.vector.scalar_tensor_tensor(
                        prod[:],
                        tt[:],
                        1.0,
                        e_neg[:],
                        op0=ALU.mult,
                        op1=ALU.mult,
                        accum_out=acc_spos[:, sl : sl + 1],
                    )
                    nc.vector.scalar_tensor_tensor(
                        prod[:],
                        tt[:],
                        1.0,
                        e_pos[:],
                        op0=ALU.mult,
                        op1=ALU.mult,
                        accum_out=acc_sepos[:, sl : sl + 1],
                    )
                    if npos_engine == "gpsimd":
                        tf = work_pool.tile([P, f_c], F32, tag="tf")
                        nc.gpsimd.tensor_scalar(
                            tf[:],
                            tt[:],
                            1.0,
                            None,
                            op0=ALU.mult,
                            op1=ALU.add,
                            accum_out=acc_npos[:, sl : sl + 1],
                        )
                    elif npos_engine == "act":
                        tf = work_pool.tile([P, f_c], F32, tag="tf")
                        nc.scalar.activation(
                            tf[:],
                            tt[:],
                            AF.Copy,
                            accum_out=acc_npos[:, sl : sl + 1],
                        )
                    else:  # dve
                        tf = work_pool.tile([P, f_c], F32, tag="tf")
                        nc.vector.tensor_scalar(
                            tf[:],
                            tt[:],
                            1.0,
                            None,
                            op0=ALU.mult,
                            op1=ALU.add,
                            accum_out=acc_npos[:, sl : sl + 1],
                        )

            # --- epilogue: combine chunk partials, per-row loss, reduce ---
            s_pos = small_pool.tile([P, n_rg], F32, tag="s_pos")
            se_pos = small_pool.tile([P, n_rg], F32, tag="se_pos")
            s_all = small_pool.tile([P, n_rg], F32, tag="s_all")
            n_pos = small_pool.tile([P, n_rg], F32, tag="n_pos")

            def chunk_reduce(dst, src):
                nc.vector.tensor_reduce(
                    dst[:],
                    src[:].rearrange("p (g c) -> p g c", c=n_ch),
                    axis=mybir.AxisListType.X,
                    op=ALU.add,
                )

            chunk_reduce(s_pos, acc_spos)
            chunk_reduce(se_pos, acc_sepos)
            chunk_reduce(s_all, acc_sall)
            chunk_reduce(n_pos, acc_npos)

            s_neg = small_pool.tile([P, n_rg], F32, tag="s_neg")
            nc.vector.tensor_tensor(
                s_neg[:], s_all[:], se_pos[:], op=ALU.subtract
            )
            numer = small_pool.tile([P, n_rg], F32, tag="numer")
            nc.vector.tensor_tensor(numer[:], s_pos[:], s_neg[:], op=ALU.mult)
            # denom = (n_pos - L) * n_pos = -(n_pos * n_neg)
            denom = small_pool.tile([P, n_rg], F32, tag="denom")
            nc.vector.scalar_tensor_tensor(
                denom[:],
                n_pos[:],
                float(cols),
                n_pos[:],
                op0=ALU.subtract,
                op1=ALU.mult,
            )
            recip = small_pool.tile([P, n_rg], F32, tag="recip")
            nc.vector.reciprocal(recip[:], denom[:])
            contrib = small_pool.tile([P, n_rg], F32, tag="contrib")
            nc.vector.tensor_tensor(contrib[:], numer[:], recip[:], op=ALU.mult)

            # partition reduction: (-1s)^T @ contrib -> [1, n_rg] (sign flips back)
            neg_ones = small_pool.tile([P, 1], F32, tag="neg_ones")
            nc.vector.memset(neg_ones[:], -1.0)
            ps = psum_pool.tile([1, n_rg], F32, tag="ps")
            nc.tensor.matmul(ps[:], neg_ones[:], contrib[:])
            res = small_pool.tile([1, 1], F32, tag="res")
            nc.vector.tensor_reduce(
                res[:], ps[:], axis=mybir.AxisListType.X, op=ALU.add
            )
            nc.sync.dma_start(out[0:1, 0:1], res[:])

    nc.compile()
    return nc


_NC_CACHE = {}


def _get_nc():
    if "nc" not in _NC_CACHE:
        _NC_CACHE["nc"] = build_bass()
    return _NC_CACHE["nc"]


def kernel(input, target):
    x = np.ascontiguousarray(np.asarray(input, dtype=np.float32))
    t = np.ascontiguousarray(np.asarray(target, dtype=np.int32))
    assert x.shape == (B, L) and t.shape == (B, L)

    nc = _get_nc()
    in_maps = [
        {
            "x": x[i * ROWS : (i + 1) * ROWS],
            "t": t[i * ROWS : (i + 1) * ROWS],
        }
        for i in range(N_CORES)
    ]
    res = run_bass_kernel_spmd(nc, in_maps, core_ids=list(range(N_CORES)))
    partials = np.array(
        [res.results[i]["out"][0, 0] for i in range(N_CORES)], dtype=np.float64
    )
    return np.float32(partials.sum())
